# revision 6
# baseline (speedup 1.0000x reference)
"""Trainium2 Bass kernel for nn_EnhancedMemoryAttentionLayer (DETR-style decoder layer).

Layer: self-attn (L=4096 tokens, D=256, H=8 heads) -> LN -> cross-attn over
M=4096 memory tokens -> LN -> FFN(2048) -> LN, post-norm residual layout.

Sharding: sequence-parallel over the 8 NeuronCores (512 queries each); K/V
computed replicated on every core, so no collectives are needed.

Per-core pipeline (matmuls in bf16 with fp32 PSUM accumulate):
  - natural 4-head packing: head h%4 at partitions 32*(h%4), chunk h//4.
    Score matmuls are K=32 row-tiled at tile_position (32*hh, 0), so the four
    heads of a group execute concurrently in the PE array (16x 32x32
    sub-arrays); each head's scores land in its own PSUM bank (separate
    tiles - PSUM accessors serialize whole-tile, so shared tiles would
    serialize the exp lanes).
  - softmax exp is split across ACT (native Exp) and DVE (Schraudolph
    fast-exp: i16 = s*A + B, bitcast to bf16; constant centered so the
    multiplicative offset is ~1; the +-3.3% spread washes out over the
    4096-key diffuse softmax - measured no effect on the final error).
    Assignment per 512-element chunk via a build-time load balancer.
  - PV matmuls are M=33 col-tiled in pairs at tile_position (0,0)/(0,64),
    accumulating into a pre-zeroed PSUM bank (a K=1 zero-matmul opens the
    bank-wide accumulation group); PV is emitted one j-round behind exp so
    exp latency never stalls the PE stream.
  - denominators ride as a ones-column in V (VW=33); the epilogue copies a
    whole pv bank to SBUF once, reciprocals the two rowsum rows, and
    broadcasts them across partitions with a K=1 fp32 matmul (no DRAM
    round-trip); normalization multiply on DVE.
  - all PSUM->SBUF egress (K/V/weight copies, relu, q/o copies) goes through
    the load balancer (ACT/DVE; gpsimd for SBUF-only work).
  - cross-attention K/V build + weight preprocessing are emitted as small
    two-phase "filler" units interleaved into the self-attention j-loop
    (compute phase this round, egress next round), so their PE matmuls run
    in PE's exp-wait bubbles; FFN weight prep fills the cross span; the
    prologue rotates its kv/weight units over all 8 PSUM banks.

Measured: relative error 7.6e-4 vs the fp32 reference; ~420 us/iteration on
8 trn2 cores (R-repeat wall-clock deltas through the axon tunnel; baseline
was 737 us). Cost-model timeline: 356 us (it serializes tile-packed matmuls,
so real HW runs the attention spans faster than modeled).
"""

import sys

sys.path.insert(0, "/opt/trn_rl_repo")

import numpy as np

L, B, D, H, M, F = 4096, 1, 256, 8, 4096, 2048
DH = D // H
P = 128
NCORES = 8
SH = L // NCORES          # queries per core
KC = D // P               # 2 contraction chunks of the model dim
FC = F // P               # 16 chunks of the FFN dim
LC = SH // P              # 4 query chunks per core
NK = L // P               # 32 key chunks (same for memory: M//P)
SCALE = 1.0 / np.sqrt(DH)
EPS = 1e-5
VW = DH + 1               # V columns per head incl. ones column (33)

# Schraudolph fast-exp constants (bf16 via int16 bits), calibrated on HW:
# i16 = trunc/round(s * FEXP_A + FEXP_B); bitcast bf16 ~= C * exp(s*SCALE)
# with C ~= 1 (centered) and +-3.3% spread.
FEXP_A = float(SCALE * 128.0 * np.log2(np.e))
FEXP_B = float(127.0 * 128 + 57.25 - 62.79)

_CACHE = {}
_LOAD_DBG = {}


def _build(zb=False, gtriv=False, reps=1):
    import concourse.bass as bass
    import concourse.mybir as mybir
    import concourse.tile as tile
    from concourse import bacc
    from concourse.masks import make_identity

    f32 = mybir.dt.float32
    f32r = mybir.dt.float32r
    bf = mybir.dt.bfloat16
    i16 = mybir.dt.int16
    Alu = mybir.AluOpType
    Act = mybir.ActivationFunctionType

    nc = bacc.Bacc("TRN2", target_bir_lowering=False, debug=False)

    def din(name, shape):
        return nc.dram_tensor(name, list(shape), f32, kind="ExternalInput")

    tgtT_d = din("tgtT", (D, L))
    qpT_d = din("qpT", (D, L))
    memT_d = din("memT", (D, M))
    posT_d = din("posT", (D, M))
    tgt_sh_d = din("tgt_sh", (SH, D))
    qp_sh_d = din("qp_sh", (SH, D))
    tgt_shT_d = din("tgt_shT", (D, SH))
    qp_shT_d = din("qp_shT", (D, SH))
    w_in_s_d = din("w_in_s", (3 * D, D))
    b_in_s_d = din("b_in_s", (3 * D,))
    w_out_s_d = din("w_out_s", (D, D))
    b_out_s_d = din("b_out_s", (D,))
    w_in_c_d = din("w_in_c", (3 * D, D))
    b_in_c_d = din("b_in_c", (3 * D,))
    w_out_c_d = din("w_out_c", (D, D))
    b_out_c_d = din("b_out_c", (D,))
    w1_d = din("w1", (F, D))
    b1_d = din("b1", (F,))
    w2_d = din("w2", (D, F))
    b2_d = din("b2", (D,))
    g1_d = din("g1", (D,))
    be1_d = din("be1", (D,))
    g2_d = din("g2", (D,))
    be2_d = din("be2", (D,))
    g3_d = din("g3", (D,))
    be3_d = din("be3", (D,))
    out_d = nc.dram_tensor("out", [SH, D], f32, kind="ExternalOutput")

    with tile.TileContext(nc) as tc:
        with (
            tc.tile_pool(name="cst", bufs=1) as cst,
            tc.tile_pool(name="dual", bufs=2) as dual,
            tc.tile_pool(name="wk", bufs=2) as wk,
            tc.tile_pool(name="pt", bufs=4) as ptp,
            tc.tile_pool(name="pt2", bufs=3) as ptp2,
            tc.tile_pool(name="scp", bufs=1, space="PSUM") as scp,
            tc.tile_pool(name="pvp", bufs=1, space="PSUM") as pvp,
            tc.tile_pool(name="pm", bufs=1, space="PSUM") as pmp,
            tc.tile_pool(name="ps", bufs=1, space="PSUM") as psp,
            tc.tile_pool(name="dram", bufs=1, space="DRAM") as dpool,
        ):
          for _rep in range(reps):
            ident = dual.tile([P, P], bf, tag="ident", name="ident")
            make_identity(nc, ident)
            epsT = cst.tile([P, 1], f32, tag="eps")
            nc.vector.memset(epsT, EPS)
            ones_row = cst.tile([1, 512], bf, tag="ones_row")
            nc.vector.memset(ones_row, 1.0)
            zrow = cst.tile([1, P], bf, tag="zrow")
            nc.vector.memset(zrow, 0.0)
            ones_f32 = cst.tile([P, P], f32, tag="ones_f32")
            nc.vector.memset(ones_f32, 1.0)

            # ---------- ACT/DVE(/gpsimd) egress load balancer ----------
            _LOAD_DBG.clear()
            # per-element ns estimates; gpsimd only eligible for SBUF-only ops
            _RATE = {"act": 0.833, "dve": 1.042, "gps": 1.80}
            eng_load = {"act": 0.0, "dve": 0.0, "gps": 0.0}

            def pick_engine(engs=("act", "dve")):
                return min(engs, key=lambda e: eng_load[e])

            def charge(eng, els):
                eng_load[eng] += els * _RATE[eng] + 175.0
                _LOAD_DBG.update(eng_load)

            def _g3(ap):
                # gpsimd tensor ops want >=3d access patterns
                return ap.rearrange("p (a b) -> p a b", a=1)

            def ecopy(out_ap, in_ap, els, sbuf=False):
                e = pick_engine(("act", "dve", "gps") if sbuf else ("act", "dve"))
                charge(e, els)
                if e == "act":
                    nc.scalar.activation(out_ap, in_ap, Act.Copy)
                elif e == "dve":
                    nc.vector.tensor_copy(out_ap, in_ap)
                else:
                    nc.gpsimd.tensor_copy(_g3(out_ap), _g3(in_ap))

            def eadd(out_ap, in0, in1, els, sbuf=True):
                e = pick_engine(("dve", "gps") if sbuf else ("dve",))
                charge(e, els)
                if e == "dve":
                    nc.vector.tensor_add(out_ap, in0, in1)
                else:
                    nc.gpsimd.tensor_add(out_ap, in0, in1)

            def erelu(out_ap, in_ap, els):
                e = pick_engine()
                charge(e, els)
                if e == "act":
                    nc.scalar.activation(out_ap, in_ap, Act.Relu)
                else:
                    nc.vector.tensor_relu(out_ap, in_ap)

            def eexp(out_ap, in_ap, els):
                e = pick_engine()
                charge(e, els)
                if e == "act":
                    nc.scalar.activation(out_ap, in_ap, Act.Exp, scale=SCALE)
                else:
                    nc.vector.tensor_scalar(
                        out_ap.bitcast(i16), in_ap, FEXP_A, FEXP_B,
                        Alu.mult, Alu.add,
                    )

            # ---------- psum bank rotation for kv/weight units ----------
            # fillers run with the single spare banks (pmp for projections,
            # psp for transposes); the prologue passes all attention banks.
            # score banks are paired: scA/scB are [P, 2, 512] two-bank tiles
            # (merged exp reads 1024 els per op; ACT on scA, DVE on scB so
            # the engines touch disjoint PSUM banks).
            FILL_BANKS = [(pmp, "m", None)]
            PRO_BANKS = ([(scp, "scA", 0), (scp, "scA", 1),
                          (scp, "scB", 0), (scp, "scB", 1),
                          (pvp, "pv0", None), (pvp, "pv1", None),
                          (pmp, "m", None)])

            def make_ps_alloc(banks):
                cnt = [0]
                def alloc():
                    pool, tg, idx = banks[cnt[0] % len(banks)]
                    cnt[0] += 1
                    if idx is None:
                        return pool.tile([P, 512], f32, tag=tg,
                                         name=f"u_{tg}")
                    t = pool.tile([P, 2, 512], f32, tag=tg, name=f"u_{tg}")
                    return t[:, idx, :]
                return alloc

            # ---------- weights: W^T in SBUF, bf16 ----------
            def load_wT_units(dram, R, C, tag, ps_alloc=None, pool=None):
                # dram (R, C) fp32 -> [P, C//P, R] bf16 (W^T), as filler
                # units; prologue-loaded tables pass pool=dual so the next
                # rep's loads overlap this rep's tail
                rt = (pool or cst).tile([P, C // P, R], bf, tag=tag,
                                        name=f"wt_{tag}")
                csz = min(512, C)
                ntr = csz // P

                def unit(rc, cs):
                    def run():
                        st = wk.tile([P, 512], f32, bufs=4, tag="ld_a",
                                     name="st_w")
                        nc.sync.dma_start(
                            st[:, 0:csz],
                            dram.ap()[rc * P : (rc + 1) * P,
                                      cs * csz : (cs + 1) * csz],
                        )
                        sb = wk.tile([P, 4, 512], bf, tag="kv_xa", bufs=3,
                                     name="sb_w")
                        nc.gpsimd.tensor_copy(
                            sb[:, 0, 0:csz].rearrange("p (a b) -> p a b", a=1),
                            st[:, 0:csz].rearrange("p (a b) -> p a b", a=1),
                        )
                        if ps_alloc is None:
                            pst = psp.tile([P, 4, P], bf, tag="s",
                                           name="pst_w")
                        else:
                            pst = ps_alloc().bitcast(bf)[:, 0:512].rearrange(
                                "p (a b) -> p a b", a=4)
                        for cq in range(ntr):
                            nc.tensor.transpose(
                                pst[:, cq, :],
                                sb[:, 0, cq * P : (cq + 1) * P],
                                ident,
                            )
                        def egress():
                            for cq in range(ntr):
                                cc = cs * ntr + cq
                                ecopy(rt[:, cc, rc * P : (rc + 1) * P],
                                      pst[:, cq, :], P)
                        return egress
                    return run

                units = [unit(rc, cs)
                         for rc in range(R // P) for cs in range(C // csz)]
                return rt, units

            def run_units(units, depth=4):
                # pipeline: run compute phases ahead, delay egress by `depth`
                pend = []
                for u in units:
                    eg = u()
                    if eg is not None:
                        pend.append(eg)
                    if len(pend) > depth:
                        pend.pop(0)()
                for eg in pend:
                    eg()

            # ---------- biases / LN params ----------
            def brow(ap1d, tag, n=D):
                st = wk.tile([1, D], f32, tag="brow_st", name="brow_st")
                nc.sync.dma_start(st[:, 0:n], ap1d[None, :])
                t = cst.tile([1, n], bf, tag=tag, name=f"br_{tag}")
                nc.vector.tensor_copy(t, st[:, 0:n])
                return t

            def per_part(ap1d, tag, n=D):
                t = cst.tile([P, n // P], f32, tag=tag, name=f"pp_{tag}")
                nc.sync.dma_start(t, ap1d.rearrange("(c p) -> p c", p=P))
                return t

            def bcast(ap1d, tag, n=D):
                t = cst.tile([P, n], f32, tag=tag, name=f"bc_{tag}")
                src = bass.AP(tensor=ap1d.tensor, offset=ap1d.offset,
                              ap=[[0, P]] + [list(x) for x in ap1d.ap])
                nc.gpsimd.dma_start(t, src)
                return t

            if zb:
                bq_s = bk_s = bv_s = bq_c = bk_c = bv_c = None
                bo_s = bo_c = b2r = b1t = None
            else:
                bq_s = brow(b_in_s_d.ap()[0:D], "bq_s")
                bk_s = brow(b_in_s_d.ap()[D : 2 * D], "bk_s")
                bv_s = brow(b_in_s_d.ap()[2 * D : 3 * D], "bv_s")
                bq_c = brow(b_in_c_d.ap()[0:D], "bq_c")
                bk_c = brow(b_in_c_d.ap()[D : 2 * D], "bk_c")
                bv_c = brow(b_in_c_d.ap()[2 * D : 3 * D], "bv_c")
                bo_s = brow(b_out_s_d.ap(), "bo_s")
                bo_c = brow(b_out_c_d.ap(), "bo_c")
                b2r = brow(b2_d.ap(), "b2r")
                b1t = per_part(b1_d.ap(), "b1t", F)
            if gtriv:
                g1b = be1b = g2b = be2b = g3b = be3b = None
            else:
                g1b = bcast(g1_d.ap(), "g1b")
                be1b = bcast(be1_d.ap(), "be1b")
                g2b = bcast(g2_d.ap(), "g2b")
                be2b = bcast(be2_d.ap(), "be2b")
                g3b = bcast(g3_d.ap(), "g3b")
                be3b = bcast(be3_d.ap(), "be3b")

            # ---------- shard-local activations ----------
            x0 = dual.tile([P, LC, D], f32, tag="x0", name="x0")
            nc.sync.dma_start(x0, tgt_sh_d.ap().rearrange("(c p) d -> p c d", p=P))
            qp_sh = dual.tile([P, LC, D], f32, tag="qp_sh", name="qp_sh")
            nc.sync.dma_start(qp_sh, qp_sh_d.ap().rearrange("(c p) d -> p c d", p=P))

            xq_shT = dual.tile([P, KC, SH], bf, tag="xT", name="xq_shT")
            for cc in range(KC):
                a = wk.tile([P, 512], f32, tag="ld_a", bufs=4, name="a_x")
                b = wk.tile([P, 512], f32, tag="ld_b", bufs=4, name="b_x")
                nc.sync.dma_start(a, tgt_shT_d.ap()[cc * P : (cc + 1) * P, :])
                nc.sync.dma_start(b, qp_shT_d.ap()[cc * P : (cc + 1) * P, :])
                eadd(xq_shT[:, cc, :], a, b, 512)

            def project_qT(xT, WT, bq, tag):
                # q^T = Wq @ xT (+bq): [P, KC, SH] bf16, natural 4-head chunks
                qT = dual.tile([P, KC, SH], bf, tag="qT", bufs=1,
                               name=f"qT_{tag}")
                for c in range(KC):
                    ps = pmp.tile([P, 512], f32, tag="m", name="ps_q")
                    for cc in range(KC):
                        nc.tensor.matmul(
                            ps[:, 0:SH],
                            lhsT=WT[:, cc, c * P : (c + 1) * P],
                            rhs=xT[:, cc, :],
                            start=(cc == 0),
                            stop=(zb and cc == KC - 1),
                        )
                    if not zb:
                        nc.tensor.matmul(
                            ps[:, 0:SH],
                            lhsT=bq[0:1, c * P : (c + 1) * P],
                            rhs=ones_row[:, 0:SH],
                            start=False,
                            stop=True,
                        )
                    ecopy(qT[:, c, :], ps[:, 0:SH], SH)
                return qT

            # ---------- K^T and V (+ones col) build, as filler units ----------
            def build_kv_units(srcT_d, addT_d, WT, bk, bv, n_tok, tag,
                               ps_alloc=None):
                # Fine-grained units so each fits a PE exp-wait bubble:
                #   D-unit: slab DMA + gpsimd add/copy (no PE)
                #   K-unit: one K-proj chunk (2 mms + egress copy)
                #   V-unit: two tokens-chunks of V-proj (4 mms + 2 copies)
                # D-units are emitted 2 slabs ahead of their consumers.
                nkc = n_tok // P
                kT = dual.tile([P, KC, n_tok], bf, tag="kT", name=f"kT_{tag}")
                V = dual.tile([P, nkc, H * VW], bf, tag="V", name=f"V_{tag}")
                ones_view = V.rearrange("p k (h w) -> p k h w", h=H)[:, :, :, DH]
                nc.gpsimd.memset(ones_view, 1.0)
                nslab = n_tok // 512
                xab = {}

                def d_unit(nn):
                    def run():
                        sl = slice(nn * 512, (nn + 1) * 512)
                        xa = wk.tile([P, KC, 512], bf, tag="kv_xa", bufs=3,
                                     name="xa_kv")
                        xb = wk.tile([P, KC, 512], bf, tag="kv_xb", bufs=3,
                                     name="xb_kv")
                        xab[nn] = (xa, xb)
                        for cc in range(KC):
                            a = wk.tile([P, 512], f32, tag="ld_a", bufs=4,
                                        name="a_kv")
                            b = wk.tile([P, 512], f32, tag="ld_b", bufs=4,
                                        name="b_kv")
                            nc.sync.dma_start(
                                a, srcT_d.ap()[cc * P : (cc + 1) * P, sl])
                            nc.sync.dma_start(
                                b, addT_d.ap()[cc * P : (cc + 1) * P, sl])
                            eadd(xa[:, cc, :], a, b, 512)
                            ecopy(xb[:, cc, :], a, 512, sbuf=True)
                    return run

                def k_unit(nn, c):
                    def run():
                        sl = slice(nn * 512, (nn + 1) * 512)
                        xa, _ = xab[nn]
                        ps = (pmp.tile([P, 512], f32, tag="m", name="ps_k")
                              if ps_alloc is None else ps_alloc())
                        for cc in range(KC):
                            nc.tensor.matmul(
                                ps,
                                lhsT=WT[:, cc, D + c * P : D + (c + 1) * P],
                                rhs=xa[:, cc, 0:512],
                                start=(cc == 0),
                                stop=(zb and cc == KC - 1),
                            )
                        if not zb:
                            nc.tensor.matmul(
                                ps,
                                lhsT=bk[0:1, c * P : (c + 1) * P],
                                rhs=ones_row,
                                start=False,
                                stop=True,
                            )
                        def egress():
                            ecopy(kT[:, c, sl], ps, 512)
                        return egress
                    return run

                def v_unit(nn, jj):
                    def run():
                        _, xb = xab[nn]
                        ps = (pmp.tile([P, 512], f32, tag="m", name="ps_v")
                              if ps_alloc is None else ps_alloc())
                        for half in range(2):
                            for cc in range(KC):
                                nc.tensor.matmul(
                                    ps[:, half * D : (half + 1) * D],
                                    lhsT=xb[:, cc, (jj * 2 + half) * P :
                                            (jj * 2 + half + 1) * P],
                                    rhs=WT[:, cc, 2 * D : 3 * D],
                                    start=(cc == 0),
                                    stop=(zb and cc == KC - 1),
                                )
                            if not zb:
                                nc.tensor.matmul(
                                    ps[:, half * D : (half + 1) * D],
                                    lhsT=ones_row[:, 0:P],
                                    rhs=bv,
                                    start=False,
                                    stop=True,
                                )
                        def egress():
                            for half in range(2):
                                j = nn * 4 + jj * 2 + half
                                vv = V[:, j, :].rearrange(
                                    "p (h w) -> p h w", h=H)[:, :, 0:DH]
                                pv_ = ps[:, half * D : (half + 1) * D].rearrange(
                                    "p (h w) -> p h w", h=H)
                                ecopy(vv, pv_, D)
                        return egress
                    return run

                units = [d_unit(0)]
                if nslab > 1:
                    units.append(d_unit(1))
                for nn in range(nslab):
                    units.append(k_unit(nn, 0))
                    units.append(k_unit(nn, 1))
                    units.append(v_unit(nn, 0))
                    units.append(v_unit(nn, 1))
                    if nn + 2 < nslab:
                        units.append(d_unit(nn + 2))
                return kT, V, units

            # ---------- attention core ----------
            def attention(kT, V, qT, WoT, bo, x_res, g_b, be_b, tag, fillers):
                # fillers: list of closures; one is consumed per j-round to
                # fill PE exp-wait bubbles with independent work.
                oT_raw = dual.tile([P, KC, SH], f32, tag="oT_raw",
                                   name="oT_raw")
                oT_n = dual.tile([P, KC, SH], bf, tag="oT_n", name="oT_n")
                fill_i = 0
                pending_egress = [None]

                for g in range(2):           # head groups (4 heads each)
                    pvs = [pvp.tile([P, 512], f32, tag=f"pv{b2}",
                                    name=f"pv{b2}_t") for b2 in range(2)]
                    # open each pv bank with a zeroing matmul so the two
                    # col-tiled heads per bank can accumulate into one group
                    for b2 in range(2):
                        nc.tensor.matmul(
                            pvs[b2],
                            lhsT=zrow[0:1, :],
                            rhs=ones_row[:, 0:512],
                            start=True, stop=False,
                            skip_group_check=True,
                        )
                    def emit_pv(j, pTs):
                        for hh in range(4):
                            h = g * 4 + hh
                            base = 64 * (hh % 2)
                            nc.tensor.matmul(
                                pvs[hh // 2][base : base + VW, :],
                                lhsT=V[:, j, h * VW : (h + 1) * VW],
                                rhs=pTs[hh],
                                start=False,
                                stop=(j == NK - 1 and hh % 2 == 1),
                                tile_position=(0, base),
                                skip_group_check=True,
                            )

                    # two 2-bank score tiles: heads 0,1 -> scA (exp on ACT),
                    # heads 2,3 -> scB (exp on DVE); one 1024-el exp per
                    # tile per round keeps the engines on disjoint banks.
                    scs2 = [scp.tile([P, 2, 512], f32, tag=t, name=f"{t}_t")
                            for t in ("scA", "scB")]
                    prev = None   # (j, pTs) one-round delay slot: PV(j-1)
                    for j in range(NK):
                        for hh in range(4):
                            hp = 32 * hh
                            nc.tensor.matmul(
                                scs2[hh // 2][:, hh % 2, :],
                                lhsT=kT[hp : hp + 32, g, j * P : (j + 1) * P],
                                rhs=qT[hp : hp + 32, g, :],
                                start=True, stop=True,
                                tile_position=(hp, 0),
                            )
                        pT2 = [ptp.tile([P, 2, 512], bf, tag=t,
                                        name=f"{t}_t")
                               for t in ("pT01", "pT23")]
                        pTs = [pT2[hh // 2][:, hh % 2, :] for hh in range(4)]
                        charge("act", 1024)
                        nc.scalar.activation(pT2[0], scs2[0], Act.Exp,
                                             scale=SCALE)
                        charge("dve", 1024)
                        nc.vector.tensor_scalar(
                            pT2[1].bitcast(i16), scs2[1], FEXP_A, FEXP_B,
                            Alu.mult, Alu.add,
                        )
                        if prev is not None:
                            emit_pv(*prev)
                        prev = (j, pTs)
                        rnd = g * NK + j
                        target = min(len(fillers),
                                     (rnd + 1) * len(fillers) // (2 * NK) + 1,
                                     fill_i + 2)
                        while fill_i < target:
                            if pending_egress[0] is not None:
                                pending_egress[0]()
                                pending_egress[0] = None
                            pending_egress[0] = fillers[fill_i]() or None
                            fill_i += 1
                    emit_pv(*prev)
                    # per-bank epilogue: one PSUM read per bank, then all
                    # further work from SBUF (recip on the two rowsum rows
                    # via a partition-strided view)
                    for b2 in range(2):
                        oh = wk.tile([P, 512], f32, tag="ld_b", bufs=4,
                                     name="oh_t")
                        ecopy(oh, pvs[b2], 512)
                        # partitions DH and 64+DH hold the two rowsum rows
                        charge("dve", 512)
                        rrs = oh[DH : DH + 1, :]
                        nc.vector.reciprocal(rrs, rrs)
                        charge("dve", 512)
                        rrs2 = oh[64 + DH : 64 + DH + 1, :]
                        nc.vector.reciprocal(rrs2, rrs2)
                        rbpt = scp.tile([P, 2, 512], f32,
                                        tag=("scA" if b2 == 0 else "scB"),
                                        name="rbp")
                        for i2 in range(2):
                            hh = b2 * 2 + i2
                            h = g * 4 + hh
                            base = 64 * i2
                            hc4, hp4 = h // 4, 32 * (h % 4)
                            nc.sync.dma_start(
                                oT_raw[hp4 : hp4 + DH, hc4, :],
                                oh[base : base + DH, :],
                            )
                            # broadcast 1/rowsum across partitions: K=1
                            # f32r matmul (ones column x reciprocal row)
                            rbp = rbpt[:, i2, :]
                            nc.tensor.matmul(
                                rbp,
                                lhsT=ones_f32[base + DH : base + DH + 1, 0:P],
                                rhs=oh[base + DH : base + DH + 1, :],
                                start=True, stop=True,
                                tile_position=(base + DH, 0),
                            )
                            charge("dve", 512)
                            nc.vector.tensor_mul(
                                oT_n[hp4 : hp4 + DH, hc4, :],
                                oT_raw[hp4 : hp4 + DH, hc4, :],
                                rbp[hp4 : hp4 + DH, :],
                            )

                # consume any leftover fillers
                if pending_egress[0] is not None:
                    pending_egress[0]()
                    pending_egress[0] = None
                while fill_i < len(fillers):
                    eg = fillers[fill_i]()
                    if eg is not None:
                        eg()
                    fill_i += 1

                # out-proj + residual + LN; yp banks reuse the sc pool
                x_next = cst.tile([P, LC, D], f32, tag=f"x_{tag}")
                yp2 = [scp.tile([P, 2, 512], f32, tag=t, name=f"yp_{t}")
                       for t in ("scA", "scB")]
                yps = [yp2[lq // 2][:, lq % 2, :] for lq in range(LC)]
                for lq in range(LC):
                    yp = yps[lq]
                    for cc in range(KC):
                        nc.tensor.matmul(
                            yp[:, 0:D],
                            lhsT=oT_n[:, cc, lq * P : (lq + 1) * P],
                            rhs=WoT[:, cc, :],
                            start=(cc == 0),
                            stop=(zb and cc == KC - 1),
                        )
                    if not zb:
                        nc.tensor.matmul(
                            yp[:, 0:D],
                            lhsT=ones_row[:, 0:P],
                            rhs=bo,
                            start=False,
                            stop=True,
                        )
                for lq in range(LC):
                    _residual_ln(yps[lq][:, 0:D], x_res[:, lq, :], g_b, be_b,
                                 x_next[:, lq, :])
                return x_next

            def _residual_ln(y_ps, x_res, g_b, be_b, out_ap):
                charge("dve", 3 * D)
                s_t = wk.tile([P, D], f32, tag="lnS", bufs=4, name="lnS")
                nc.vector.tensor_add(s_t, y_ps, x_res)
                stats = wk.tile([P, 6], f32, tag="lnStats", name="lnStats")
                nc.vector.bn_stats(stats, s_t)
                mv = wk.tile([P, 2], f32, tag="lnMv", name="lnMv")
                nc.vector.bn_aggr(mv, stats)
                std = wk.tile([P, 1], f32, tag="lnStd", name="lnStd")
                nc.scalar.activation(std, mv[:, 1:2], Act.Sqrt, bias=epsT)
                rstd = wk.tile([P, 1], f32, tag="lnRstd", name="lnRstd")
                nc.vector.reciprocal(rstd, std)
                nc.vector.tensor_scalar(
                    out_ap, s_t, mv[:, 0:1], rstd, Alu.subtract, Alu.mult
                )
                if not gtriv:
                    nc.vector.tensor_mul(out_ap, out_ap, g_b)
                    nc.vector.tensor_add(out_ap, out_ap, be_b)

            # ---------- prologue: self weights, self kv, self q ----------
            pro_alloc = make_ps_alloc(PRO_BANKS)
            WT_s, wts_units = load_wT_units(w_in_s_d, 3 * D, D, "wt_s",
                                            ps_alloc=pro_alloc, pool=dual)
            WoT_s, wos_units = load_wT_units(w_out_s_d, D, D, "wot_s",
                                             ps_alloc=pro_alloc, pool=dual)
            kT_s, V_s, kvs_units = build_kv_units(
                tgtT_d, qpT_d, WT_s, bk_s, bv_s, L, "s", ps_alloc=pro_alloc)
            # stream the first kv slabs while the weight chains run
            run_units(kvs_units[:2])
            run_units(wts_units)
            run_units(wos_units)
            run_units(kvs_units[2:])
            qT_s = project_qT(xq_shT, WT_s, bq_s, "s")

            # fillers for the self span: cross weights + cross kv build
            WT_c, wtc_units = load_wT_units(w_in_c_d, 3 * D, D, "wt_c")
            WoT_c, woc_units = load_wT_units(w_out_c_d, D, D, "wot_c")
            kT_c, V_c, kvc_units = build_kv_units(
                memT_d, posT_d, WT_c, bk_c, bv_c, M, "c")
            self_fill = wtc_units + woc_units + kvc_units

            x1 = attention(kT_s, V_s, qT_s, WoT_s, bo_s, x0,
                           g1b, be1b, "self", self_fill)

            # q input for cross attn: x1 + query_pos, transposed on-chip
            xq2T = dual.tile([P, KC, SH], bf, tag="xT", name="xq2T")
            tr_alloc = make_ps_alloc(PRO_BANKS)
            for lq in range(LC):
                xq2 = wk.tile([P, D], bf, tag="xq2", bufs=4, name="xq2")
                charge("gps", 256)
                nc.gpsimd.tensor_add(_g3(xq2), _g3(x1[:, lq, :]),
                                     _g3(qp_sh[:, lq, :]))
                pst2 = tr_alloc().bitcast(bf)[:, 0:512].rearrange(
                    "p (a b) -> p a b", a=4)
                for cc in range(KC):
                    nc.tensor.transpose(
                        pst2[:, cc, :], xq2[:, cc * P : (cc + 1) * P], ident
                    )
                for cc in range(KC):
                    ecopy(xq2T[:, cc, lq * P : (lq + 1) * P],
                          pst2[:, cc, :], P)
            qT_c = project_qT(xq2T, WT_c, bq_c, "c")

            # fillers for the cross span: FFN weight preprocessing
            W1T, w1_units = load_wT_units(w1_d, F, D, "w1t")
            W2T, w2_units = load_wT_units(w2_d, D, F, "w2t")
            cross_fill = w1_units + w2_units

            x2 = attention(kT_c, V_c, qT_c, WoT_c, bo_c, x1,
                           g2b, be2b, "cross", cross_fill)

            # ---------- FFN ----------
            x2T = dual.tile([P, KC, SH], bf, tag="xT", name="x2T")
            tr_alloc2 = make_ps_alloc(PRO_BANKS)
            for lq in range(LC):
                x2b = wk.tile([P, D], bf, tag="xq2", bufs=4, name="x2b")
                charge("gps", 256)
                nc.gpsimd.tensor_copy(_g3(x2b), _g3(x2[:, lq, :]))
                pst3 = tr_alloc2().bitcast(bf)[:, 0:512].rearrange(
                    "p (a b) -> p a b", a=4)
                for cc in range(KC):
                    nc.tensor.transpose(
                        pst3[:, cc, :], x2b[:, cc * P : (cc + 1) * P], ident
                    )
                for cc in range(KC):
                    ecopy(x2T[:, cc, lq * P : (lq + 1) * P],
                          pst3[:, cc, :], P)

            # y accumulators: one bank per query chunk (scp pool, 4 banks)
            ypf2 = [scp.tile([P, 2, 512], f32, tag=t, name=f"ypf_{t}")
                    for t in ("scA", "scB")]
            ypfs = [ypf2[lq // 2][:, lq % 2, :] for lq in range(LC)]

            hbs = [pvp.tile([P, 512], f32, tag=f"pv{b2}",
                            name=f"hb{b2}") for b2 in range(2)]
            for q in range(4):
                hq = wk.tile([P, 4, 512], bf, tag="kv_xa", bufs=3, name="hq")
                for f4 in range(4):
                    fc = q * 4 + f4
                    psf = hbs[fc % 2]
                    for cc in range(KC):
                        nc.tensor.matmul(
                            psf,
                            lhsT=W1T[:, cc, fc * P : (fc + 1) * P],
                            rhs=x2T[:, cc, :],
                            start=(cc == 0),
                            stop=(cc == KC - 1),
                        )
                    if zb:
                        erelu(hq[:, f4, :], psf, 512)
                    else:
                        nc.scalar.activation(
                            hq[:, f4, :], psf, Act.Relu,
                            bias=b1t[:, fc : fc + 1],
                        )
                    for lq in range(LC):
                        nc.tensor.matmul(
                            ypfs[lq][:, 0:D],
                            lhsT=hq[:, f4, lq * P : (lq + 1) * P],
                            rhs=W2T[:, fc, :],
                            start=(fc == 0),
                            stop=(zb and fc == FC - 1),
                        )
            if not zb:
                for lq in range(LC):
                    nc.tensor.matmul(
                        ypfs[lq][:, 0:D],
                        lhsT=ones_row[:, 0:P],
                        rhs=b2r,
                        start=False,
                        stop=True,
                    )

            out_t = cst.tile([P, LC, D], f32, tag="out_t")
            out_v = out_d.ap().rearrange("(c p) d -> p c d", p=P)
            for lq in range(LC):
                _residual_ln(ypfs[lq][:, 0:D], x2[:, lq, :],
                             g3b, be3b, out_t[:, lq, :])
                nc.sync.dma_start(out_v[:, lq, :], out_t[:, lq, :])

    nc.compile()
    nc._eng_load_dbg = dict(eng_load)
    return nc


_FLAGS = {"zb": False, "gtriv": False}


def _get_nc(reps=1):
    key = ("nc", _FLAGS["zb"], _FLAGS["gtriv"], reps)
    if key not in _CACHE:
        _CACHE[key] = _build(_FLAGS["zb"], _FLAGS["gtriv"], reps)
    return _CACHE[key]


def _in_maps(inputs):
    c32 = lambda a: np.ascontiguousarray(np.asarray(a), dtype=np.float32)
    tgt = c32(inputs["tgt"]).reshape(L, D)
    qp = c32(inputs["query_pos"]).reshape(L, D)
    mem = c32(inputs["memory"]).reshape(M, D)
    pos = c32(inputs["pos"]).reshape(M, D)
    shared = {
        "tgtT": np.ascontiguousarray(tgt.T),
        "qpT": np.ascontiguousarray(qp.T),
        "memT": np.ascontiguousarray(mem.T),
        "posT": np.ascontiguousarray(pos.T),
        "w_in_s": c32(inputs["w_in_self"]),
        "b_in_s": c32(inputs["b_in_self"]),
        "w_out_s": c32(inputs["w_out_self"]),
        "b_out_s": c32(inputs["b_out_self"]),
        "w_in_c": c32(inputs["w_in_cross"]),
        "b_in_c": c32(inputs["b_in_cross"]),
        "w_out_c": c32(inputs["w_out_cross"]),
        "b_out_c": c32(inputs["b_out_cross"]),
        "w1": c32(inputs["w1"]),
        "b1": c32(inputs["b1"]),
        "w2": c32(inputs["w2"]),
        "b2": c32(inputs["b2"]),
        "g1": c32(inputs["g1"]),
        "be1": c32(inputs["be1"]),
        "g2": c32(inputs["g2"]),
        "be2": c32(inputs["be2"]),
        "g3": c32(inputs["g3"]),
        "be3": c32(inputs["be3"]),
    }
    maps = []
    for c in range(NCORES):
        sl = slice(c * SH, (c + 1) * SH)
        m = dict(shared)
        m["tgt_sh"] = np.ascontiguousarray(tgt[sl])
        m["qp_sh"] = np.ascontiguousarray(qp[sl])
        m["tgt_shT"] = np.ascontiguousarray(tgt[sl].T)
        m["qp_shT"] = np.ascontiguousarray(qp[sl].T)
        maps.append(m)
    return maps


def _make_runner(reps=1):
    import jax
    from jax.experimental.shard_map import shard_map
    from jax.sharding import Mesh, PartitionSpec

    from concourse import bass2jax, mybir

    nc = _get_nc(reps)
    bass2jax.install_neuronx_cc_hook()
    partition_name = nc.partition_id_tensor.name if nc.partition_id_tensor else None
    in_names, out_names, out_avals, zero_shapes = [], [], [], []
    for alloc in nc.m.functions[0].allocations:
        if not isinstance(alloc, mybir.MemoryLocationSet):
            continue
        name = alloc.memorylocations[0].name
        if alloc.kind == "ExternalInput":
            if name != partition_name:
                in_names.append(name)
        elif alloc.kind == "ExternalOutput":
            shape = tuple(alloc.tensor_shape)
            dtype = mybir.dt.np(alloc.dtype)
            out_avals.append(jax.core.ShapedArray(shape, dtype))
            out_names.append(name)
            zero_shapes.append((shape, dtype))
    n_params = len(in_names)
    all_names = list(in_names + out_names)
    if partition_name is not None:
        all_names.append(partition_name)
    donate = tuple(range(n_params, n_params + len(out_names)))

    def _body(*args):
        operands = list(args)
        if partition_name is not None:
            operands.append(bass2jax.partition_id_tensor())
        outs = bass2jax._bass_exec_p.bind(
            *operands,
            out_avals=tuple(out_avals),
            in_names=tuple(all_names),
            out_names=tuple(out_names),
            lowering_input_output_aliases=(),
            sim_require_finite=True,
            sim_require_nnan=True,
            nc=nc,
        )
        return tuple(outs)

    devices = jax.devices()[:NCORES]
    mesh = Mesh(np.asarray(devices), ("core",))
    in_specs = (PartitionSpec("core"),) * (n_params + len(out_names))
    out_specs = (PartitionSpec("core"),) * len(out_names)
    sharded = jax.jit(
        shard_map(_body, mesh=mesh, in_specs=in_specs, out_specs=out_specs,
                  check_rep=False),
        donate_argnums=donate,
        keep_unused=True,
    )
    return {"fn": sharded, "in_names": in_names, "out_names": out_names,
            "zero_shapes": zero_shapes, "mesh": mesh}


def _runner(reps=1):
    key = ("runner", _FLAGS["zb"], _FLAGS["gtriv"], reps)
    if key not in _CACHE:
        _CACHE[key] = _make_runner(reps)
    return _CACHE[key]


def _concat_inputs(maps, in_names):
    return [
        np.concatenate([maps[c][n] for c in range(NCORES)], axis=0)
        for n in in_names
    ]


def _zeros(zero_shapes):
    return [np.zeros((NCORES * s[0], *s[1:]), d) for s, d in zero_shapes]


def _set_flags(inputs):
    z = lambda k: not np.any(np.asarray(inputs[k]))
    o = lambda k: np.all(np.asarray(inputs[k]) == 1.0)
    _FLAGS["zb"] = all(z(k) for k in (
        "b_in_self", "b_out_self", "b_in_cross", "b_out_cross", "b1", "b2"))
    _FLAGS["gtriv"] = (all(o(k) for k in ("g1", "g2", "g3"))
                       and all(z(k) for k in ("be1", "be2", "be3")))


def kernel(**inputs):
    _set_flags(inputs)
    r = _runner()
    maps = _in_maps(inputs)
    concat_in = _concat_inputs(maps, r["in_names"])
    outs = r["fn"](*concat_in, *_zeros(r["zero_shapes"]))
    oi = r["out_names"].index("out")
    out = np.asarray(outs[oi]).astype(np.float32)
    return out.reshape(L, B, D)

